# revision 1
# baseline (speedup 1.0000x reference)
"""Trainium2 Bass kernel for causal average pooling (downsampling).

Reference op: out[b, i, d] = mean(x[b, :(i+1)*4, d]) over the time axis,
for x of shape (8, 8192, 512) f32 -> out (8, 2048, 512) f32.

Strategy
--------
Data-parallel over batch: one batch per NeuronCore (8 cores), no
cross-core communication.

Per core the math is, for each channel d independently, a prefix sum
over time sampled every SF=4 steps, scaled by 1/(4(i+1)).  We lay the
data out as [channel partitions, time free-dim] (the host pre-transposes
each batch, which is pure layout) and use the hardware prefix scan
`tensor_tensor_scan` on the vector engine:

    state = (data0[t] + state) + data1[t]

Feeding data0 = x[:, 0::2] and data1 = x[:, 1::2] gives the cumulative
sum over PAIRS: cs2[:, j] = sum(x[:, :2j+2]).  Output i of the reference
needs sum(x[:, :4i+4]) = cs2[:, 2i+1]: a strided gather of the odd
columns times a 1/(4(i+1)) table (an 8 KB host row, replicated to all
128 partitions by an idle-PE ones[1,128].T @ row matmul so the table
never crosses the DMA fabric at full size).

Each 128-channel tile's time axis is cut into pieces which are scanned
INDEPENDENTLY (initial=0.0 — chaining through an AP initial measured
~2.3 us slower per scan).  A piece's missing carry (total of the earlier
pieces, maintained as a [128,1] running column) is folded into its
output op for free with scalar_tensor_tensor:
    out = (cs_local + carry) * recip.
The kernel is DMA-fabric-bound (~430 GB/s shared by loads+stores), so
the last tile is tapered into small pieces to shrink the serial tail
(last-load receipt -> scan -> out -> store -> receipt).

Pipeline per core (xT [512 chan, 8192 time], 4 channel tiles, x tiles
triple-buffered to ride out HBM receipt-latency jitter):
  SP ring:   recip row (8 KB) then x piece loads (2 MiB, tapered tail)
  PE+ACT:    recip broadcast matmul + PSUM->SBUF copies (once, idle units)
  ACT ring:  per-piece output stores
  DVE:       per piece: scan + gather*recip (TT / STT), carry columns

Written in raw Bass (not Tile): the walrus build in this container
enforces at most ONE semaphore wait per hardware instruction, so all
cross-engine waits are standalone wait_ge ops.  Each load gets its own
semaphore because completions of back-to-back DMAs on one HWDGE ring
are unordered.
"""

import sys

if "/opt/trn_rl_repo" not in sys.path:
    sys.path.insert(0, "/opt/trn_rl_repo")

import numpy as np

import concourse.bass as bass
import concourse.mybir as mybir
from concourse.bass_utils import run_bass_kernel_spmd

P = 128           # SBUF partitions
SF = 4            # pooling factor
B, L, D = 8, 8192, 512
N_CORES = 8


def _pieces(n_ct, length):
    """Per-tile piece boundaries in x columns. Pieces are half a tile
    (2 MiB) except the final tile, which tapers down so the serial tail
    after the last load (receipt -> scan -> out -> store) is short."""
    halves = [(0, length // 2), (length // 2, length)]
    if length < 4096:
        return [halves] * n_ct
    # First tile ramps up (0.5 MiB first piece) so the DVE chain — which
    # paces the kernel end-to-end — starts as early as possible.
    ramp = [
        (0, length // 8),
        (length // 8, length // 4),
        (length // 4, length // 2),
        (length // 2, 3 * length // 4),
        (3 * length // 4, length),
    ]
    taper = [
        (0, length // 2),
        (length // 2, 3 * length // 4),
        (3 * length // 4, 7 * length // 8),
        (7 * length // 8, 15 * length // 16),
        (15 * length // 16, 31 * length // 32),
        (31 * length // 32, length),
    ]
    return [ramp] + [halves] * (n_ct - 2) + [taper]


def build_bass(d=D, length=L):
    half = length // 2          # scan steps per tile (pairs)
    out_len = length // SF
    n_ct = d // P
    assert d % P == 0 and length % (2 * SF * 8) == 0

    nc = bass.Bass()
    xT = nc.dram_tensor("xT", [d, length], mybir.dt.float32, kind="ExternalInput")
    # recip row plus 128 trailing 1.0s (the PE broadcast lhsT) in one input.
    recip = nc.dram_tensor(
        "recip", [1, out_len + P], mybir.dt.float32, kind="ExternalInput"
    )
    outT = nc.dram_tensor(
        "outT", [d, out_len], mybir.dt.float32, kind="ExternalOutput"
    )

    pieces = _pieces(n_ct, length)
    n_loads = sum(len(p) for p in pieces)

    # DVE op index bookkeeping (s_cmp is incremented by every DVE op).
    cmp_val = 0
    scan_val = [[None] * len(pieces[ct]) for ct in range(n_ct)]  # scan done
    out_val = [[None] * len(pieces[ct]) for ct in range(n_ct)]   # out op done

    with (
        nc.sbuf_tensor([P, length], mybir.dt.float32) as xt0,
        nc.sbuf_tensor([P, length], mybir.dt.float32) as xt1,
        nc.sbuf_tensor([P, length], mybir.dt.float32) as xt2,
        nc.sbuf_tensor([P, half], mybir.dt.float32) as cs0,
        nc.sbuf_tensor([P, half], mybir.dt.float32) as cs1,
        nc.sbuf_tensor([1, out_len + P], mybir.dt.float32) as rrow,
        nc.sbuf_tensor([1, P], mybir.dt.float32) as ones,
        nc.sbuf_tensor([1, 1], mybir.dt.float32) as scr,
        nc.psum_tensor([P, out_len], mybir.dt.float32) as rps,
        nc.sbuf_tensor([P, out_len], mybir.dt.float32) as rt,
        nc.sbuf_tensor([P, n_ct], mybir.dt.float32) as runc,
        nc.sbuf_tensor([P, n_ct, out_len], mybir.dt.float32) as ot,
        nc.semaphore("s_rrow") as s_rrow,
        nc.semaphore("s_ones") as s_ones,
        nc.semaphore("s_ps") as s_ps,
        nc.semaphore("s_rt") as s_rt,
        nc.semaphore("s_cmp") as s_cmp,
        nc.semaphore("s_out") as s_out,
        nc.Block() as block,
    ):
        n_banks = (out_len + 511) // 512
        bank_cols = min(512, out_len)
        s_xs = [nc.alloc_semaphore(f"s_x{i}") for i in range(n_loads)]
        xts = [xt0, xt1, xt2]
        n_xb = len(xts)
        css = [cs0, cs1]

        # ---- plan the DVE op order so cross-engine wait values are known ---
        # Default per piece: [(run-col update?), scan, out].  The run-col
        # update only depends on EARLIER pieces' scans, so it runs before
        # this piece's scan and stays off the critical scan->out tail chain.
        # Tile 0 runs its first two scans back-to-back BEFORE their outs:
        # the outs need the recip table, whose on-chip broadcast chain is
        # still in flight when the first small ramp pieces land.
        def _tile_order(ct):
            n_p = len(pieces[ct])
            if ct == 0 and n_p >= 3:
                order = [("scan", 0), ("scan", 1), ("out", 0), ("out", 1)]
                for p in range(2, n_p):
                    order += [("runc", p), ("scan", p), ("out", p)]
                return order
            order = []
            for p in range(n_p):
                if p >= 2:
                    order.append(("runc", p))
                order += [("scan", p), ("out", p)]
            return order

        for ct in range(n_ct):
            for kind, p in _tile_order(ct):
                cmp_val += 1
                if kind == "scan":
                    scan_val[ct][p] = cmp_val
                elif kind == "out":
                    out_val[ct][p] = cmp_val

        @block.sync
        def _(sync):
            # x loads only on the SP HWDGE ring (the recip row rides the ACT
            # ring so the first x byte isn't delayed by its issue slot).
            li = 0
            for ct in range(n_ct):
                for p, (xs, xe) in enumerate(pieces[ct]):
                    if ct >= n_xb:
                        # buffer WAR: last scan of tile ct-n_xb whose region
                        # overlaps this piece must be done with the buffer.
                        last = max(
                            pp for pp, (ps, pe) in enumerate(pieces[ct - n_xb])
                            if ps < xe and pe > xs
                        )
                        sync.wait_ge(s_cmp, scan_val[ct - n_xb][last])
                    sync.dma_start(
                        out=xts[ct % n_xb][:, xs:xe],
                        in_=xT[ct * P:(ct + 1) * P, xs:xe],
                    ).then_inc(s_xs[li], 16)
                    li += 1

        @block.gpsimd
        def _(gpsimd):
            nc.gpsimd.memset(ones[:, :], 1.0).then_inc(s_ones, 1)

        @block.tensor
        def _(tensor):
            # Broadcast the 8 KB recip row to all 128 partitions on the
            # (otherwise idle) PE: ones[1,128].T @ rrow[1,bank] replicates the
            # row into PSUM, so the table never crosses the DMA fabric at
            # full size.  One matmul per PSUM bank (N<=512).
            tensor.wait_ge(s_rrow, 16)
            ones_ap = rrow[:, out_len:out_len + P]
            for k in range(n_banks):
                nc.tensor.matmul(
                    rps[:, k * bank_cols:(k + 1) * bank_cols],
                    ones_ap,
                    rrow[:, k * bank_cols:(k + 1) * bank_cols],
                    start=True,
                    stop=True,
                ).then_inc(s_ps, 1)

        @block.vector
        def _(vector):
            cval = 0
            rt_banks_waited = [0]
            li_base = 0
            for ct in range(n_ct):
                cs = css[ct % 2][:, :]
                xtile = xts[ct % n_xb]
                for kind, p in _tile_order(ct):
                    xs, xe = pieces[ct][p]
                    c0, c1 = xs // 2, xe // 2    # cs (pair) columns
                    o0, o1 = xs // 4, xe // 4    # output columns
                    if kind == "runc":
                        # carry column: total of pieces 0..p-1.  Only depends
                        # on earlier scans, so it runs BEFORE this piece's
                        # scan (off the critical scan->out tail chain).
                        vector.wait_ge(s_cmp, scan_val[ct][p - 1])
                        prev_end = pieces[ct][p - 1][1] // 2
                        if p == 2:
                            first_end = pieces[ct][0][1] // 2
                            nc.vector.tensor_add(
                                runc[:, ct:ct + 1],
                                cs[:, first_end - 1:first_end],
                                cs[:, prev_end - 1:prev_end],
                            ).then_inc(s_cmp, 1)
                        else:
                            nc.vector.tensor_add(
                                runc[:, ct:ct + 1],
                                runc[:, ct:ct + 1],
                                cs[:, prev_end - 1:prev_end],
                            ).then_inc(s_cmp, 1)
                        cval += 1
                    elif kind == "scan":
                        vector.wait_ge(s_xs[li_base + p], 16)
                        if ct >= 2:
                            # cs WAW vs tile ct-2's final out; trivially
                            # satisfied by DVE order, for the race checker.
                            vector.wait_ge(s_cmp, out_val[ct - 2][-1])
                        xv = xtile[:, xs:xe].rearrange(
                            "p (t two) -> p t two", two=2
                        )
                        nc.vector.tensor_tensor_scan(
                            cs[:, c0:c1],
                            xv[:, :, 0],
                            xv[:, :, 1],
                            0.0,
                            mybir.AluOpType.add,
                            mybir.AluOpType.add,
                        ).then_inc(s_cmp, 1)
                        cval += 1
                        assert cval == scan_val[ct][p]
                    else:  # out
                        # scan -> out RAW on the same engine; for the checker.
                        vector.wait_ge(s_cmp, scan_val[ct][p])
                        # The recip table is only needed by OUT ops, and only
                        # the copied slices covering this piece's output
                        # columns — the first out starts as soon as the first
                        # half-bank is ready.
                        need = (o1 + bank_cols - 1) // bank_cols
                        if need > rt_banks_waited[0]:
                            vector.wait_ge(s_rt, need)
                            rt_banks_waited[0] = need
                        csv = cs[:, c0:c1].rearrange(
                            "p (t two) -> p t two", two=2
                        )
                        o_ap = ot[:, ct, o0:o1]
                        r_ap = rt[:, o0:o1]
                        if p == 0:
                            nc.vector.tensor_mul(
                                o_ap, csv[:, :, 1], r_ap
                            ).then_inc(s_cmp, 1)
                        elif p == 1:
                            # carry is just piece 0's total column.
                            nc.vector.scalar_tensor_tensor(
                                o_ap, csv[:, :, 1], cs[:, c0 - 1:c0], r_ap,
                                mybir.AluOpType.add, mybir.AluOpType.mult,
                            ).then_inc(s_cmp, 1)
                        else:
                            nc.vector.scalar_tensor_tensor(
                                o_ap, csv[:, :, 1], runc[:, ct:ct + 1], r_ap,
                                mybir.AluOpType.add, mybir.AluOpType.mult,
                            ).then_inc(s_cmp, 1)
                        cval += 1
                        assert cval == out_val[ct][p]
                li_base += len(pieces[ct])

        @block.scalar
        def _(scalar):
            # 8 KB recip row + output stores on the ACT HWDGE ring; the
            # PSUM->SBUF copies of the broadcast recip table run on the idle
            # ACT ALU.
            scalar.dma_start(out=rrow[:, :], in_=recip[:, :]).then_inc(s_rrow, 16)
            # Dummy 1-element copy: demand-loads the ACT function table NOW
            # so the real PSUM->SBUF copies below don't pay the ~1.3 us
            # table-load on the recip-table critical path.
            scalar.wait_ge(s_ones, 1)
            nc.scalar.copy(scr[:, :], ones[:, 0:1])
            for k in range(n_banks):
                scalar.wait_ge(s_ps, k + 1)
                nc.scalar.copy(
                    rt[:, k * bank_cols:(k + 1) * bank_cols],
                    rps[:, k * bank_cols:(k + 1) * bank_cols],
                ).then_inc(s_rt, 1)
            n_stores = 0
            for ct in range(n_ct):
                for p, (xs, xe) in enumerate(pieces[ct]):
                    o0, o1 = xs // 4, xe // 4
                    scalar.wait_ge(s_cmp, out_val[ct][p])
                    scalar.dma_start(
                        out=outT[ct * P:(ct + 1) * P, o0:o1],
                        in_=ot[:, ct, o0:o1],
                    ).then_inc(s_out, 16)
                    n_stores += 1
            # Outputs must be in HBM before the kernel exits.
            scalar.wait_ge(s_out, 16 * n_stores)

    return nc


def _recip_row(out_len):
    r = 1.0 / (SF * np.arange(1, out_len + 1, dtype=np.float64))
    row = np.concatenate([r.astype(np.float32), np.ones(P, np.float32)])
    return row.reshape(1, out_len + P)


def kernel(x: np.ndarray) -> np.ndarray:
    b, length, d = x.shape
    out_len = length // SF
    # One batch per core, channels on partitions: host-side transpose is
    # pure layout so every DMA in the kernel is contiguous.
    xT = np.ascontiguousarray(np.swapaxes(np.asarray(x, dtype=np.float32), 1, 2))
    recip = _recip_row(out_len)
    in_maps = [{"xT": xT[i], "recip": recip} for i in range(b)]
    nc = build_bass(d=d, length=length)
    res = run_bass_kernel_spmd(nc, in_maps, core_ids=list(range(b)))
    outT = np.stack([res.results[i]["outT"] for i in range(b)])
    return np.ascontiguousarray(np.swapaxes(outT, 1, 2))



# revision 5
# speedup vs baseline: 1.0833x; 1.0833x over previous
"""Trainium2 Bass kernel for causal average pooling (downsampling).

Reference op: out[b, i, d] = mean(x[b, :(i+1)*4, d]) over the time axis,
for x of shape (8, 8192, 512) f32 -> out (8, 2048, 512) f32.

Strategy
--------
Data-parallel over batch: one batch per NeuronCore (8 cores), no
cross-core communication.

The kernel is HBM-bound (~358 GB/s per core), so both HBM legs run in
bf16: the host casts x to bf16 (tolerance is 2e-2; bf16-in/bf16-out
measures ~4.6e-3) halving the load bytes, and the output is stored as
bf16 and upcast on the host, halving the store bytes.  Per-core traffic
drops 21->10.5 MB.

Per core the data is laid out [channel partitions, time free-dim] (host
pre-transposes, pure layout).  The hardware prefix scan is a serial
recurrence (~2.7 ns/step regardless of width), so scanning raw pairs
would cost 43 us; instead DVE first reduces time 4x on its fast
throughput path (~0.27 ns/col):

    s2 = x[0::2] + x[1::2]            # TT add, bf16
    s4 = s2[0::2] + s2[1::2]          # TT add, bf16 (4-block sums)
    cs = pair-scan(s4_even, s4_odd)   # fp32 states at every 8th x col

cs[k] covers 8(k+1) x columns = every SECOND output.  The two phases:

    out_odd[k]  = (cs[k] + carry) * 1/(8(k+1))          # STT
    tmp[k]      = (s4_even[k] + carry) + cs[k-1]        # STT (cs has a
    out_even[k] = tmp[k] * 1/(4(2k+1))                  # TT   zero col 0)

with carry = total of the tile's earlier pieces, maintained as a
[128,1] running column (scan initial stays 0.0 per piece: chaining the
scan through an AP initial measured ~2.3 us slower).

The two recip tables live de-interleaved in one host row
[even 1024 | odd 1024 | ones 128]; the 128 trailing ones are the lhsT
of a ones[1,128].T @ row matmul that broadcasts the row to all 128
partitions on the otherwise-idle PE (so the table never crosses the DMA
fabric at full size), one matmul per PSUM bank, in bank order 0,2,1,3
so the first slices of BOTH halves are ready first.

Each 128-channel tile's time axis is cut into pieces loaded separately;
the first tile ramps up (small first piece so the DVE chain starts
early) and the last tile tapers down (small last pieces to shrink the
serial tail: last-load receipt -> pools -> scan -> outs -> store).

Pipeline per core (xT [512 chan, 8192 time] bf16, 4 channel tiles,
x tiles triple-buffered):
  SP ring:   x piece loads (tapered)
  ACT ring:  recip row (8.7 KB) then per-piece output stores
  PE+ACT:    recip broadcast matmuls + PSUM->SBUF copies (idle units)
  DVE:       per piece: s2, s4 pools; pair-scan; tmp/out_even/out_odd
  GpSimd:    one-time memsets (cs zero cols, ACT-warmup operand)

Written in raw Bass (not Tile): the walrus build in this container
enforces at most ONE semaphore wait per hardware instruction, so all
cross-engine waits are standalone wait_ge ops.  Each load gets its own
semaphore because completions of back-to-back DMAs on one HWDGE ring
are unordered.
"""

import sys

if "/opt/trn_rl_repo" not in sys.path:
    sys.path.insert(0, "/opt/trn_rl_repo")

import ml_dtypes
import numpy as np

import concourse.bass as bass
import concourse.mybir as mybir
from concourse.bass_utils import run_bass_kernel_spmd

P = 128           # SBUF partitions
SF = 4            # pooling factor
B, L, D = 8, 8192, 512
N_CORES = 8
BF16 = ml_dtypes.bfloat16


def _pieces(n_ct, length):
    """Per-tile piece boundaries in x columns (all multiples of 8)."""
    halves = [(0, length // 2), (length // 2, length)]
    if length < 4096:
        return [halves] * n_ct
    # First tile ramps up so the DVE chain starts as early as possible.
    ramp = [
        (0, length // 8),
        (length // 8, length // 4),
        (length // 4, length // 2),
        (length // 2, 3 * length // 4),
        (3 * length // 4, length),
    ]
    # Last tile tapers down so the post-last-load serial tail is short.
    taper = [
        (0, length // 2),
        (length // 2, 3 * length // 4),
        (3 * length // 4, 7 * length // 8),
        (7 * length // 8, 15 * length // 16),
        (15 * length // 16, 31 * length // 32),
        (31 * length // 32, length),
    ]
    return [ramp] + [halves] * (n_ct - 2) + [taper]


def build_bass(d=D, length=L):
    out_len = length // SF
    half_out = out_len // 2        # scan steps per full tile
    n_ct = d // P
    assert d % P == 0 and length % (2 * SF * 8) == 0

    nc = bass.Bass()
    xT = nc.dram_tensor("xT", [d, length], mybir.dt.bfloat16, kind="ExternalInput")
    # [recip_even out_len/2 | recip_odd out_len/2 | ones P] in one input.
    recip = nc.dram_tensor(
        "recip", [1, out_len + P], mybir.dt.float32, kind="ExternalInput"
    )
    outT = nc.dram_tensor(
        "outT", [d, out_len], mybir.dt.bfloat16, kind="ExternalOutput"
    )

    pieces = _pieces(n_ct, length)
    n_loads = sum(len(p) for p in pieces)

    # DVE op index bookkeeping (s_cmp is incremented by every DVE op).
    pool_val = [[None] * len(pieces[ct]) for ct in range(n_ct)]  # s4 pool done
    scan_val = [[None] * len(pieces[ct]) for ct in range(n_ct)]  # scan done
    out_val = [[None] * len(pieces[ct]) for ct in range(n_ct)]   # last out done

    # Per-piece op chain: s2 pool, s4 pool, (carry update), scan, tmp,
    # out_even, out_odd.  Tile 0 front-runs pieces 0+1 through their scans
    # before the first out ops: the outs need the recip table, whose
    # on-chip broadcast chain is still in flight when the first small
    # ramp pieces land.
    def _tile_order(ct):
        n_p = len(pieces[ct])
        if ct == 0 and n_p >= 3:
            order = [
                ("pools", 0), ("scan", 0),
                ("pools", 1), ("scan", 1),
                ("tmp", 0), ("oe", 0), ("oo", 0),
                ("tmp", 1), ("oe", 1), ("oo", 1),
            ]
            for p in range(2, n_p):
                order += [
                    ("pools", p), ("runc", p), ("scan", p),
                    ("tmp", p), ("oe", p), ("oo", p),
                ]
            return order
        order = []
        for p in range(n_p):
            order.append(("pools", p))
            if p >= 2:
                order.append(("runc", p))
            order += [("scan", p), ("tmp", p), ("oe", p), ("oo", p)]
        return order

    cmp_val = 0
    for ct in range(n_ct):
        for kind, p in _tile_order(ct):
            if kind == "pools":
                cmp_val += 2
                pool_val[ct][p] = cmp_val
            else:
                cmp_val += 1
                if kind == "scan":
                    scan_val[ct][p] = cmp_val
                elif kind == "oo":
                    out_val[ct][p] = cmp_val

    from contextlib import ExitStack

    with ExitStack() as stack:
        en = stack.enter_context
        xt0 = en(nc.sbuf_tensor([P, length], mybir.dt.bfloat16))
        xt1 = en(nc.sbuf_tensor([P, length], mybir.dt.bfloat16))
        xt2 = en(nc.sbuf_tensor([P, length], mybir.dt.bfloat16))
        s2 = en(nc.sbuf_tensor([P, length // 2], mybir.dt.bfloat16))
        s4 = en(nc.sbuf_tensor([P, length // 4], mybir.dt.bfloat16))
        cs0 = en(nc.sbuf_tensor([P, 1 + half_out], mybir.dt.float32))
        cs1 = en(nc.sbuf_tensor([P, 1 + half_out], mybir.dt.float32))
        tmp = en(nc.sbuf_tensor([P, half_out], mybir.dt.float32))
        rrow = en(nc.sbuf_tensor([1, out_len + P], mybir.dt.float32))
        ones = en(nc.sbuf_tensor([1, P], mybir.dt.float32))
        scr = en(nc.sbuf_tensor([1, 1], mybir.dt.float32))
        rps = en(nc.psum_tensor([P, out_len], mybir.dt.float32))
        rt = en(nc.sbuf_tensor([P, out_len], mybir.dt.float32))
        runc = en(nc.sbuf_tensor([P, n_ct], mybir.dt.float32))
        ot = en(nc.sbuf_tensor([P, n_ct, out_len], mybir.dt.bfloat16))
        s_rrow = en(nc.semaphore("s_rrow"))
        s_ones = en(nc.semaphore("s_ones"))
        s_z = en(nc.semaphore("s_z"))
        s_ps = en(nc.semaphore("s_ps"))
        s_rt = en(nc.semaphore("s_rt"))
        s_cmp = en(nc.semaphore("s_cmp"))
        s_out = en(nc.semaphore("s_out"))
        block = en(nc.Block())
        n_banks = (out_len + 511) // 512
        bank_cols = min(512, out_len)
        # Banks in order 0,2,1,3: low slices of the even AND odd halves
        # first, so tile 0's first out ops unblock as early as possible.
        bank_order = [0, 2, 1, 3][:n_banks] if n_banks == 4 else list(range(n_banks))
        rt_pos = {k: j + 1 for j, k in enumerate(bank_order)}  # bank -> s_rt count
        s_xs = [nc.alloc_semaphore(f"s_x{i}") for i in range(n_loads)]
        xts = [xt0, xt1, xt2]
        n_xb = len(xts)
        css = [cs0, cs1]

        @block.sync
        def _(sync):
            # x loads only on the SP HWDGE ring (the recip row rides the ACT
            # ring so the first x byte isn't delayed by its issue slot).
            li = 0
            for ct in range(n_ct):
                for p, (xs, xe) in enumerate(pieces[ct]):
                    if ct >= n_xb:
                        # buffer WAR: last s4 pool of tile ct-n_xb whose
                        # region overlaps this piece must be done reading.
                        last = max(
                            pp for pp, (ps, pe) in enumerate(pieces[ct - n_xb])
                            if ps < xe and pe > xs
                        )
                        sync.wait_ge(s_cmp, pool_val[ct - n_xb][last])
                    sync.dma_start(
                        out=xts[ct % n_xb][:, xs:xe],
                        in_=xT[ct * P:(ct + 1) * P, xs:xe],
                    ).then_inc(s_xs[li], 16)
                    li += 1

        @block.gpsimd
        def _(gpsimd):
            nc.gpsimd.memset(ones[:, :], 1.0).then_inc(s_ones, 1)
            nc.gpsimd.memset(cs0[:, 0:1], 0.0).then_inc(s_z, 1)
            nc.gpsimd.memset(cs1[:, 0:1], 0.0).then_inc(s_z, 1)

        @block.tensor
        def _(tensor):
            # Broadcast the recip row to all 128 partitions on the idle PE:
            # ones[1,128].T @ rrow[1,bank] replicates the row into PSUM.
            tensor.wait_ge(s_rrow, 16)
            ones_ap = rrow[:, out_len:out_len + P]
            for k in bank_order:
                nc.tensor.matmul(
                    rps[:, k * bank_cols:(k + 1) * bank_cols],
                    ones_ap,
                    rrow[:, k * bank_cols:(k + 1) * bank_cols],
                    start=True,
                    stop=True,
                ).then_inc(s_ps, 1)

        @block.vector
        def _(vector):
            cval = 0
            rt_seen = [0]
            z_waited = [False]

            def need_rt(pos):
                if pos > rt_seen[0]:
                    vector.wait_ge(s_rt, pos)
                    rt_seen[0] = pos

            li_base = 0
            for ct in range(n_ct):
                xtile = xts[ct % n_xb]
                n8s = [(xe - xs) // 8 for (xs, xe) in pieces[ct]]
                for kind, p in _tile_order(ct):
                    xs, xe = pieces[ct][p]
                    n8 = n8s[p]
                    q0, q1 = xs // 4, xe // 4      # s4 columns
                    e8 = xs // 8                   # phase-block start
                    cs = css[p % 2]
                    carry = runc[:, ct:ct + 1]
                    if kind == "pools":
                        vector.wait_ge(s_xs[li_base + p], 16)
                        xv = xtile[:, xs:xe].rearrange(
                            "p (t two) -> p t two", two=2
                        )
                        nc.vector.tensor_add(
                            s2[:, xs // 2:xe // 2], xv[:, :, 0], xv[:, :, 1]
                        ).then_inc(s_cmp, 1)
                        sv = s2[:, xs // 2:xe // 2].rearrange(
                            "p (t two) -> p t two", two=2
                        )
                        nc.vector.tensor_add(
                            s4[:, q0:q1], sv[:, :, 0], sv[:, :, 1]
                        ).then_inc(s_cmp, 1)
                        cval += 2
                        assert cval == pool_val[ct][p]
                    elif kind == "runc":
                        # carry column: total of pieces 0..p-1 (only depends
                        # on earlier scans; runs before this piece's scan to
                        # stay off the critical scan->out tail chain).
                        if p == 2:
                            nc.vector.tensor_add(
                                carry,
                                css[0][:, n8s[0]:n8s[0] + 1],
                                css[1][:, n8s[1]:n8s[1] + 1],
                            ).then_inc(s_cmp, 1)
                        else:
                            nc.vector.tensor_add(
                                carry,
                                carry,
                                css[(p - 1) % 2][:, n8s[p - 1]:n8s[p - 1] + 1],
                            ).then_inc(s_cmp, 1)
                        cval += 1
                    elif kind == "scan":
                        s4v = s4[:, q0:q1].rearrange(
                            "p (t two) -> p t two", two=2
                        )
                        nc.vector.tensor_tensor_scan(
                            cs[:, 1:1 + n8],
                            s4v[:, :, 0],
                            s4v[:, :, 1],
                            0.0,
                            mybir.AluOpType.add,
                            mybir.AluOpType.add,
                        ).then_inc(s_cmp, 1)
                        cval += 1
                        assert cval == scan_val[ct][p]
                    elif kind == "tmp":
                        # tmp[k] = (s4_even[k] + carry) + cs[k-1]
                        if not z_waited[0]:
                            vector.wait_ge(s_z, 2)
                            z_waited[0] = True
                        s4e = s4[:, q0:q1].rearrange(
                            "p (t two) -> p t two", two=2
                        )[:, :, 0]
                        if p == 0:
                            nc.vector.tensor_add(
                                tmp[:, :n8], s4e, cs[:, 0:n8]
                            ).then_inc(s_cmp, 1)
                        else:
                            car = (
                                css[0][:, n8s[0]:n8s[0] + 1] if p == 1 else carry
                            )
                            nc.vector.scalar_tensor_tensor(
                                tmp[:, :n8], s4e, car, cs[:, 0:n8],
                                mybir.AluOpType.add, mybir.AluOpType.add,
                            ).then_inc(s_cmp, 1)
                        cval += 1
                    elif kind == "oe":
                        need_rt(rt_pos[(e8 + n8 - 1) // bank_cols])
                        ov = ot[:, ct, xs // 4:xe // 4].rearrange(
                            "p (k two) -> p k two", two=2
                        )
                        nc.vector.tensor_mul(
                            ov[:, :, 0], tmp[:, :n8], rt[:, e8:e8 + n8]
                        ).then_inc(s_cmp, 1)
                        cval += 1
                    else:  # oo
                        o8 = half_out + e8
                        need_rt(rt_pos[(o8 + n8 - 1) // bank_cols])
                        ov = ot[:, ct, xs // 4:xe // 4].rearrange(
                            "p (k two) -> p k two", two=2
                        )
                        if p == 0:
                            nc.vector.tensor_mul(
                                ov[:, :, 1], cs[:, 1:1 + n8], rt[:, o8:o8 + n8]
                            ).then_inc(s_cmp, 1)
                        else:
                            car = (
                                css[0][:, n8s[0]:n8s[0] + 1] if p == 1 else carry
                            )
                            nc.vector.scalar_tensor_tensor(
                                ov[:, :, 1], cs[:, 1:1 + n8], car,
                                rt[:, o8:o8 + n8],
                                mybir.AluOpType.add, mybir.AluOpType.mult,
                            ).then_inc(s_cmp, 1)
                        cval += 1
                        assert cval == out_val[ct][p]
                li_base += len(pieces[ct])

        @block.scalar
        def _(scalar):
            # Recip row + output stores on the ACT HWDGE ring; the
            # PSUM->SBUF copies of the broadcast recip table run on the
            # idle ACT ALU.
            scalar.dma_start(out=rrow[:, :], in_=recip[:, :]).then_inc(s_rrow, 16)
            # Dummy 1-element copy: demand-loads the ACT function table NOW
            # so the real PSUM->SBUF copies below don't pay the ~1.3 us
            # table-load on the recip-table critical path.
            scalar.wait_ge(s_ones, 1)
            nc.scalar.copy(scr[:, :], ones[:, 0:1])
            for j, k in enumerate(bank_order):
                scalar.wait_ge(s_ps, j + 1)
                nc.scalar.copy(
                    rt[:, k * bank_cols:(k + 1) * bank_cols],
                    rps[:, k * bank_cols:(k + 1) * bank_cols],
                ).then_inc(s_rt, 1)
            n_stores = 0
            for ct in range(n_ct):
                for p, (xs, xe) in enumerate(pieces[ct]):
                    o0, o1 = xs // 4, xe // 4
                    scalar.wait_ge(s_cmp, out_val[ct][p])
                    scalar.dma_start(
                        out=outT[ct * P:(ct + 1) * P, o0:o1],
                        in_=ot[:, ct, o0:o1],
                    ).then_inc(s_out, 16)
                    n_stores += 1
            # Outputs must be in HBM before the kernel exits.
            scalar.wait_ge(s_out, 16 * n_stores)

    return nc


def _recip_row(out_len):
    # [recip_even | recip_odd | ones]: even outputs 2k scale 1/(4(2k+1)),
    # odd outputs 2k+1 scale 1/(8(k+1)).
    k = np.arange(out_len // 2, dtype=np.float64)
    even = 1.0 / (4.0 * (2.0 * k + 1.0))
    odd = 1.0 / (8.0 * (k + 1.0))
    row = np.concatenate(
        [even.astype(np.float32), odd.astype(np.float32), np.ones(P, np.float32)]
    )
    return row.reshape(1, out_len + P)


def prep_in_maps(x):
    b, length, d = x.shape
    # One batch per core, channels on partitions; bf16 halves the HBM bytes.
    xT = np.swapaxes(np.asarray(x, dtype=np.float32), 1, 2).astype(BF16)
    recip = _recip_row(length // SF)
    return [{"xT": xT[i], "recip": recip} for i in range(b)]


def post(results, b):
    outT = np.stack([np.asarray(results[i]["outT"]) for i in range(b)])
    return np.ascontiguousarray(
        np.swapaxes(outT.astype(np.float32), 1, 2)
    )


def kernel(x: np.ndarray) -> np.ndarray:
    b, length, d = x.shape
    in_maps = prep_in_maps(x)
    nc = build_bass(d=d, length=length)
    res = run_bass_kernel_spmd(nc, in_maps, core_ids=list(range(b)))
    return post(res.results, b)
